# revision 6
# baseline (speedup 1.0000x reference)
"""Trainium2 Bass kernel for nn_PoolingModule_27616639713602.

Reference computation (B=32, S=4096, D=1024, H=16 heads, R=64 latents):
    xq    = (x @ wq) * R**-0.5                  per-token query
    attn  = softmax(einsum('nhd,rhd', xq, kv))  latent attention (per head)
    out   = einsum('nhr,rhd', attn, kv) @ wo
    h     = gelu(out @ w1 + b1);  out = h @ w2 + b2 + out
    y     = mean over S per batch               -> [B, D]

Strategy:
  * Data-parallel over 8 cores: 4 batches (16384 tokens) per core.
  * Algebra: the per-head einsums are folded into dense 1024x1024 matmuls
    using block-diagonal expansions of kv_latent:
        scores = x @ (scale * wq @ Wblk)        (Wblk[h*hd+d, h*R+r] = kv[r,h,d])
        out1   = attn_flat @ (Wblk2 @ wo)       (Wblk2[h*R+r, h*hd+d] = kv[r,h,d])
    so the whole chain is 4 dense [*,1024]x[1024,1024] matmuls + softmax + gelu.
  * Transposed dataflow: activations live as [D, tokens] in SBUF so each
    matmul's PSUM output drains directly into the next matmul's rhs operand.
    Weights are lhsT (stationary), tokens stream as the moving operand (N=512).
  * Softmax over R=64 per head in transposed layout: exp on ScalarE;
    per-head sums via matmul with a block-ones matrix; reciprocal on VectorE;
    broadcast of the reciprocal across partitions via a second tiny matmul;
    normalization as a VectorE multiply.
  * Segment mean: free-axis reductions of the PSUM results (no final
    activation materialization); residual + b2 folded in via linearity.
  * fp16 activations/weights on the PE (f32 PSUM accumulation).
"""

import numpy as np

import concourse.bass as bass
import concourse.mybir as mybir
import concourse.tile as tile
from concourse import bacc
from concourse.masks import make_identity

N_HEADS = 16
R = 64
D = 1024
HD = D // N_HEADS
P = 128
NCH = D // P          # 8 column/row chunks of 128
TOK = 512             # tokens per block (matmul moving free dim)
S = 4096              # tokens per segment (batch)
N_CORES = 8

f32 = mybir.dt.float32
f16 = mybir.dt.float16

AX = mybir.AxisListType
AF = mybir.ActivationFunctionType


def build_nc(nb, s_blocks=S // TOK, debug=False, act_fn=AF.Gelu):
    """Build the per-core Bass program.

    nb: number of segments (batches) this core handles.
    s_blocks: 512-token blocks per segment (8 for S=4096).
    """
    nblocks = nb * s_blocks
    ntok = nblocks * TOK
    nc = bacc.Bacc("TRN2", target_bir_lowering=False, debug=debug)

    x_d = nc.dram_tensor("x", [ntok, D], f32, kind="ExternalInput").ap()
    wqs_d = nc.dram_tensor("wqs", [D, D], f16, kind="ExternalInput").ap()
    wow_d = nc.dram_tensor("wow", [D, D], f16, kind="ExternalInput").ap()
    w1_d = nc.dram_tensor("w1", [D, D], f16, kind="ExternalInput").ap()
    w2_d = nc.dram_tensor("w2", [D, D], f16, kind="ExternalInput").ap()
    b1_d = nc.dram_tensor("b1t", [P, NCH], f32, kind="ExternalInput").ap()
    onesb_d = nc.dram_tensor("onesb", [D, N_HEADS], f16, kind="ExternalInput").ap()
    onest_d = nc.dram_tensor("onest", [N_HEADS, D], f16, kind="ExternalInput").ap()
    out_d = nc.dram_tensor("outT", [P, nb, NCH], f32, kind="ExternalOutput").ap()

    with tile.TileContext(nc) as tc:
        with (
            tc.tile_pool(name="singles", bufs=1) as singles,
            tc.tile_pool(name="xn_pool", bufs=8) as xn_pool,
            tc.tile_pool(name="acts", bufs=2) as acts,
            tc.tile_pool(name="smalls", bufs=2) as smalls,
            tc.tile_pool(name="rbp", bufs=2) as rbp,
            tc.tile_pool(name="mmps", bufs=4, space="PSUM") as mmps,
            tc.tile_pool(name="tpps", bufs=2, space="PSUM") as tpps,
            tc.tile_pool(name="smps", bufs=2, space="PSUM") as smps,
        ):
            # ---- resident constants -------------------------------------
            identity = singles.tile([P, P], f32)
            make_identity(nc, identity)

            wqs_sb = singles.tile([P, NCH, D], f16)
            nc.sync.dma_start(out=wqs_sb, in_=wqs_d.rearrange("(k p) d -> p k d", p=P))
            wow_sb = singles.tile([P, NCH, D], f16)
            nc.sync.dma_start(out=wow_sb, in_=wow_d.rearrange("(k p) d -> p k d", p=P))
            w1_sb = singles.tile([P, NCH, D], f16)
            nc.sync.dma_start(out=w1_sb, in_=w1_d.rearrange("(k p) d -> p k d", p=P))
            w2_sb = singles.tile([P, NCH, D], f16)
            nc.sync.dma_start(out=w2_sb, in_=w2_d.rearrange("(k p) d -> p k d", p=P))
            b1_sb = singles.tile([P, NCH], f32)
            nc.sync.dma_start(out=b1_sb, in_=b1_d)
            onesb_sb = singles.tile([P, NCH, N_HEADS], f16)
            nc.sync.dma_start(
                out=onesb_sb, in_=onesb_d.rearrange("(k p) h -> p k h", p=P)
            )
            onest_sb = singles.tile([N_HEADS, NCH, P], f16)
            nc.sync.dma_start(
                out=onest_sb, in_=onest_d.rearrange("h (c p) -> h c p", p=P)
            )
            out_stage = singles.tile([P, nb, NCH], f32)

            # per-block state threaded from the front stage to the back stage
            state = {}
            rbs = {}

            def front(blk):
                """Load+transpose x, MM1 (scores), exp, softmax sums/recip."""
                # x natural [tok, D] -> transposed fp16 [D, tok] in 128-chunks
                xT = acts.tile([P, NCH, TOK], f16, tag="xT")
                for st in range(TOK // P):
                    xn = xn_pool.tile([P, D], f32, tag="xn")
                    n0 = blk * TOK + st * P
                    nc.sync.dma_start(out=xn, in_=x_d[n0 : n0 + P, :])
                    for c in range(NCH):
                        pt = tpps.tile([P, P], f32, tag="pt")
                        nc.tensor.transpose(pt, xn[:, c * P : (c + 1) * P], identity)
                        nc.vector.tensor_copy(xT[:, c, st * P : (st + 1) * P], pt)

                eT = acts.tile([P, NCH, TOK], f16, tag="eT")
                sums_ps = smps.tile([N_HEADS, TOK], f32, tag="sums")
                for c in range(NCH):
                    ps = mmps.tile([P, TOK], f32, tag="ps")
                    for k in range(NCH):
                        nc.tensor.matmul(
                            ps,
                            wqs_sb[:, k, c * P : (c + 1) * P],
                            xT[:, k, :],
                            start=(k == 0),
                            stop=(k == NCH - 1),
                        )
                    nc.scalar.activation(eT[:, c, :], ps, AF.Exp)
                    nc.tensor.matmul(
                        sums_ps,
                        onesb_sb[:, c, :],
                        eT[:, c, :],
                        start=(c == 0),
                        stop=(c == NCH - 1),
                    )
                recip = smalls.tile([N_HEADS, TOK], f32, tag="recip")
                nc.vector.reciprocal(recip, sums_ps)
                recip16 = smalls.tile([N_HEADS, TOK], f16, tag="recip16")
                nc.scalar.copy(recip16, recip)
                state[blk] = (eT, recip16)

            def back(blk):
                """Normalize, MM3 (wo), MM4 (w1+gelu), MM5 (w2), reductions."""
                eT, recip16 = state.pop(blk)
                bat, bslot = divmod(blk, s_blocks)
                if bslot == 0:
                    rbs[bat] = rbp.tile(
                        [P, s_blocks * 2 * NCH], f32, tag="rb", name=f"rb{bat}"
                    )
                rb = rbs[bat]

                aT = acts.tile([P, NCH, TOK], f16, tag="aT")
                for c in range(NCH):
                    bc = mmps.tile([P, TOK], f32, tag="ps")
                    nc.tensor.matmul(bc, onest_sb[:, c, :], recip16, start=True, stop=True)
                    nc.vector.tensor_mul(aT[:, c, :], eT[:, c, :], bc)

                o1T = acts.tile([P, NCH, TOK], f16, tag="o1T")
                for c in range(NCH):
                    ps = mmps.tile([P, TOK], f32, tag="ps")
                    for k in range(NCH):
                        nc.tensor.matmul(
                            ps,
                            wow_sb[:, k, c * P : (c + 1) * P],
                            aT[:, k, :],
                            start=(k == 0),
                            stop=(k == NCH - 1),
                        )
                    iB = bslot * 2 * NCH + 2 * c + 1
                    nc.scalar.activation(
                        o1T[:, c, :], ps, AF.Copy, accum_out=rb[:, iB : iB + 1]
                    )

                h1T = acts.tile([P, NCH, TOK], f16, tag="h1T")
                for c in range(NCH):
                    ps = mmps.tile([P, TOK], f32, tag="ps")
                    for k in range(NCH):
                        nc.tensor.matmul(
                            ps,
                            w1_sb[:, k, c * P : (c + 1) * P],
                            o1T[:, k, :],
                            start=(k == 0),
                            stop=(k == NCH - 1),
                        )
                    nc.scalar.activation(
                        h1T[:, c, :], ps, act_fn, bias=b1_sb[:, c : c + 1]
                    )

                for c in range(NCH):
                    ps = mmps.tile([P, TOK], f32, tag="ps")
                    for k in range(NCH):
                        nc.tensor.matmul(
                            ps,
                            w2_sb[:, k, c * P : (c + 1) * P],
                            h1T[:, k, :],
                            start=(k == 0),
                            stop=(k == NCH - 1),
                        )
                    iA = bslot * 2 * NCH + 2 * c
                    nc.vector.reduce_sum(rb[:, iA : iA + 1], ps, axis=AX.X)

                if bslot == s_blocks - 1:
                    bsum = smalls.tile([P, NCH], f32, tag="bsum")
                    view = rb.rearrange("p (b c t) -> p c b t", c=NCH, t=2)
                    nc.vector.reduce_sum(bsum, view, axis=AX.XY)
                    nc.vector.tensor_scalar_mul(
                        out_stage[:, bat, :], bsum, 1.0 / (s_blocks * TOK)
                    )
                    del rbs[bat]

            # software pipeline: front(b+1) overlaps back(b)
            for blk in range(nblocks + 1):
                if blk < nblocks:
                    front(blk)
                if blk > 0:
                    back(blk - 1)

            nc.sync.dma_start(out=out_d, in_=out_stage)

    nc.compile()
    return nc


def fold_weights(wq, kv_latent, wo, w1, b1, w2):
    """Host-side algebraic folding (float64 for accuracy)."""
    kv = kv_latent.astype(np.float64).reshape(R, N_HEADS, HD)
    blk = np.zeros((N_HEADS, HD, N_HEADS, R))
    blk2 = np.zeros((N_HEADS, R, N_HEADS, HD))
    for h in range(N_HEADS):
        blk[h, :, h, :] = kv[:, h, :].T
        blk2[h, :, h, :] = kv[:, h, :]
    Wblk = blk.reshape(D, D)
    Wblk2 = blk2.reshape(D, D)
    wqs = (wq.astype(np.float64) @ Wblk) * (R ** -0.5)
    wow = Wblk2 @ wo.astype(np.float64)

    ones_b = np.zeros((N_HEADS, R, N_HEADS))
    for h in range(N_HEADS):
        ones_b[h, :, h] = 1.0
    onesb = ones_b.reshape(D, N_HEADS)

    return {
        "wqs": np.ascontiguousarray(wqs.astype(np.float16)),
        "wow": np.ascontiguousarray(wow.astype(np.float16)),
        "w1": np.ascontiguousarray(w1.astype(np.float16)),
        "w2": np.ascontiguousarray(w2.astype(np.float16)),
        "b1t": np.ascontiguousarray(
            b1.astype(np.float32).reshape(NCH, P).T
        ),
        "onesb": np.ascontiguousarray(onesb.astype(np.float16)),
        "onest": np.ascontiguousarray(onesb.T.astype(np.float16)),
    }


_NC_CACHE = {}
LAST_RESULTS = None


def kernel(x, wq, kv_latent, wo, w1, b1, w2, b2):
    from concourse.bass_utils import run_bass_kernel_spmd

    x = np.asarray(x, dtype=np.float32)
    B, S_, D_ = x.shape
    assert D_ == D and S_ == S and B % N_CORES == 0
    nb = B // N_CORES

    shared = fold_weights(
        np.asarray(wq), np.asarray(kv_latent), np.asarray(wo),
        np.asarray(w1), np.asarray(b1), np.asarray(w2),
    )

    key = (nb,)
    if key not in _NC_CACHE:
        _NC_CACHE[key] = build_nc(nb)
    nc = _NC_CACHE[key]

    in_maps = []
    for c in range(N_CORES):
        m = dict(shared)
        m["x"] = np.ascontiguousarray(
            x[c * nb : (c + 1) * nb].reshape(nb * S, D)
        )
        in_maps.append(m)

    res = run_bass_kernel_spmd(nc, in_maps, core_ids=list(range(N_CORES)))
    global LAST_RESULTS
    LAST_RESULTS = res
    outs = []
    for r in res.results:
        outs.append(r["outT"].transpose(1, 2, 0).reshape(nb, D))
    out = np.concatenate(outs, axis=0) + np.asarray(b2, dtype=np.float32)[None, :]
    return out.astype(np.float32)


# revision 29
# speedup vs baseline: 20593.8438x; 20593.8438x over previous
"""Trainium2 Bass kernel for nn_PoolingModule_27616639713602.

Reference computation (B=32, S=4096, D=1024, H=16 heads, R=64 latents):
    xq    = (x @ wq) * R**-0.5                  per-token query
    attn  = softmax(einsum('nhd,rhd', xq, kv))  latent attention (per head)
    out   = einsum('nhr,rhd', attn, kv) @ wo
    h     = gelu(out @ w1 + b1);  out = h @ w2 + b2 + out
    y     = mean over S per batch               -> [B, D]

Strategy:
  * Data-parallel over 8 cores: 4 batches (16384 tokens) per core.
  * Algebra: the per-head einsums are folded into dense 1024x1024 matmuls
    using block-diagonal expansions of kv_latent:
        scores = x @ (scale * wq @ Wblk)        (Wblk[h*hd+d, h*R+r] = kv[r,h,d])
        out1   = attn_flat @ (Wblk2 @ wo)       (Wblk2[h*R+r, h*hd+d] = kv[r,h,d])
    so the whole chain is 4 dense [*,1024]x[1024,1024] matmuls + softmax + gelu.
  * Transposed dataflow: activations live as [D, tokens] in SBUF so each
    matmul's PSUM output drains directly into the next matmul's rhs operand.
    Weights are lhsT (stationary), tokens stream as the moving operand (N=512).
  * Softmax over R=64 per head in transposed layout: exp on ScalarE;
    per-head sums via matmul with a block-ones matrix; reciprocal on VectorE;
    broadcast of the reciprocal across partitions via a second tiny matmul;
    normalization as a VectorE multiply.
  * Segment mean: free-axis reductions of the PSUM results (no final
    activation materialization); residual + b2 folded in via linearity.
  * fp16 activations/weights on the PE (f32 PSUM accumulation).
"""

import numpy as np

import concourse.bass as bass
import concourse.mybir as mybir
import concourse.tile as tile
from concourse import bacc

N_HEADS = 16
R = 64
D = 1024
HD = D // N_HEADS
P = 128
NCH = D // P          # 8 column/row chunks of 128
TOK = 512             # tokens per block (matmul moving free dim)
S = 4096              # tokens per segment (batch)
N_CORES = 8

f32 = mybir.dt.float32
f16 = mybir.dt.float16

AX = mybir.AxisListType
AF = mybir.ActivationFunctionType


def build_nc(nb, s_blocks=S // TOK, debug=False, act_fn=AF.Gelu, repeat=1,
             tmode="pe"):
    """Build the per-core Bass program.

    nb: number of segments (batches) this core handles.
    s_blocks: 512-token blocks per segment (8 for S=4096).
    """
    nblocks = nb * s_blocks
    ntok = nblocks * TOK
    nc = bacc.Bacc("TRN2", target_bir_lowering=False, debug=debug)

    x_d = nc.dram_tensor("x", [ntok, D], f32, kind="ExternalInput").ap()
    wqs_d = nc.dram_tensor("wqs", [D, D], f16, kind="ExternalInput").ap()
    wow_d = nc.dram_tensor("wow", [D, D], f16, kind="ExternalInput").ap()
    w1_d = nc.dram_tensor("w1", [D, D], f16, kind="ExternalInput").ap()
    w2_d = nc.dram_tensor("w2", [D, D], f16, kind="ExternalInput").ap()
    b1_d = nc.dram_tensor("b1t", [P, NCH], f32, kind="ExternalInput").ap()
    onesb_d = nc.dram_tensor("onesb", [D, N_HEADS], f16, kind="ExternalInput").ap()
    onest_d = nc.dram_tensor("onest", [N_HEADS, D], f16, kind="ExternalInput").ap()
    out_d = nc.dram_tensor("outT", [P, nb, NCH], f32, kind="ExternalOutput").ap()

    import contextlib

    with tile.TileContext(nc) as tc:
        with contextlib.ExitStack() as est:
            singles = est.enter_context(tc.tile_pool(name="singles", bufs=1))
            xn_pool = est.enter_context(tc.tile_pool(name="xn_pool", bufs=4))
            acts = est.enter_context(tc.tile_pool(name="acts", bufs=2))
            smalls = est.enter_context(tc.tile_pool(name="smalls", bufs=2))
            rbp = est.enter_context(tc.tile_pool(name="rbp", bufs=2))
            mmps = est.enter_context(
                tc.tile_pool(
                    name="mmps", bufs=(6 if tmode == "dma" else 4), space="PSUM"
                )
            )
            smps = est.enter_context(tc.tile_pool(name="smps", bufs=2, space="PSUM"))
            if tmode == "dma":
                x16_pool = est.enter_context(tc.tile_pool(name="x16_pool", bufs=6))
            else:
                tpps = est.enter_context(
                    tc.tile_pool(name="tpps", bufs=2, space="PSUM")
                )

            # ---- resident constants -------------------------------------
            if tmode == "pe":
                identity = singles.tile([P, P], f32)
                from concourse.masks import make_identity
                make_identity(nc, identity)
            # wqs + onesb first: they gate block 0's MM1. The rest are only
            # needed by back(0), ~60us later, so they load behind front(0)'s x.
            wqs_sb = singles.tile([P, NCH, D], f16)
            nc.sync.dma_start(out=wqs_sb, in_=wqs_d.rearrange("(k p) d -> p k d", p=P))
            onesb_sb = singles.tile([P, NCH, N_HEADS], f16)
            nc.sync.dma_start(
                out=onesb_sb, in_=onesb_d.rearrange("(k p) h -> p k h", p=P)
            )

            def load_rest():
                wow_sb = singles.tile([P, NCH, D], f16, name="wow_sb")
                nc.sync.dma_start(
                    out=wow_sb, in_=wow_d.rearrange("(k p) d -> p k d", p=P)
                )
                w1_sb = singles.tile([P, NCH, D], f16, name="w1_sb")
                nc.sync.dma_start(
                    out=w1_sb, in_=w1_d.rearrange("(k p) d -> p k d", p=P)
                )
                w2_sb = singles.tile([P, NCH, D], f16, name="w2_sb")
                nc.sync.dma_start(
                    out=w2_sb, in_=w2_d.rearrange("(k p) d -> p k d", p=P)
                )
                b1_sb = singles.tile([P, NCH], f32, name="b1_sb")
                nc.sync.dma_start(out=b1_sb, in_=b1_d)
                onest_sb = singles.tile([N_HEADS, NCH, P], f16, name="onest_sb")
                nc.sync.dma_start(
                    out=onest_sb, in_=onest_d.rearrange("h (c p) -> h c p", p=P)
                )
                return wow_sb, w1_sb, w2_sb, b1_sb, onest_sb

            out_stage = singles.tile([P, nb, NCH], f32)

            # per-block state threaded from the front stage to the back stage
            state = {}
            rbs = {}
            rest = {}

            def do_load_rest():
                wow_sb, w1_sb, w2_sb, b1_sb, onest_sb = load_rest()
                rest.update(
                    wow=wow_sb, w1=w1_sb, w2=w2_sb, b1=b1_sb, onest=onest_sb
                )

            def front(blk):
                """Load+transpose x, MM1 (scores), exp, softmax sums/recip."""
                # x natural [tok, D] f32 -> fp16 -> DMA-xbar transpose to [D, tok]
                xT = acts.tile([P, NCH, TOK], f16, tag="xT")
                for st in range(TOK // P):
                    xn = xn_pool.tile([P, D], f32, tag="xn")
                    n0 = blk * TOK + st * P
                    nc.sync.dma_start(out=xn, in_=x_d[n0 : n0 + P, :])
                    if tmode == "dma":
                        x16 = x16_pool.tile([P, D], f16, tag="x16")
                        nc.vector.tensor_copy(x16, xn)
                        for c in range(NCH):
                            nc.scalar.dma_start_transpose(
                                out=xT[:, c, st * P : (st + 1) * P],
                                in_=x16[:, c * P : (c + 1) * P],
                            )
                    else:
                        for c in range(NCH):
                            pt = tpps.tile([P, P], f32, tag="pt", name="pt")
                            nc.tensor.transpose(
                                pt, xn[:, c * P : (c + 1) * P], identity
                            )
                            nc.vector.tensor_copy(
                                xT[:, c, st * P : (st + 1) * P], pt
                            )

                eT = acts.tile([P, NCH, TOK], f16, tag="eT")
                sums_ps = smps.tile([N_HEADS, TOK], f32, tag="sums")
                for c in range(NCH):
                    ps = mmps.tile([P, TOK], f32, tag="ps")
                    for k in range(NCH):
                        nc.tensor.matmul(
                            ps,
                            wqs_sb[:, k, c * P : (c + 1) * P],
                            xT[:, k, :],
                            start=(k == 0),
                            stop=(k == NCH - 1),
                        )
                    nc.scalar.activation(eT[:, c, :], ps, AF.Exp)
                    nc.tensor.matmul(
                        sums_ps,
                        onesb_sb[:, c, :],
                        eT[:, c, :],
                        start=(c == 0),
                        stop=(c == NCH - 1),
                    )
                recip = smalls.tile([N_HEADS, TOK], f32, tag="recip")
                nc.vector.reciprocal(recip, sums_ps)
                recip16 = smalls.tile([N_HEADS, TOK], f16, tag="recip16")
                nc.scalar.copy(recip16, recip)
                state[blk] = (eT, recip16)

            def back(blk):
                """Normalize, MM3 (wo), MM4 (w1+gelu), MM5 (w2), reductions."""
                eT, recip16 = state.pop(blk)
                bat, bslot = divmod(blk, s_blocks)
                if bslot == 0:
                    rbs[bat] = rbp.tile(
                        [P, s_blocks * 2 * NCH], f32, tag="rb", name=f"rb{bat}"
                    )
                rb = rbs[bat]

                wow_sb, w1_sb, w2_sb = rest["wow"], rest["w1"], rest["w2"]
                b1_sb, onest_sb = rest["b1"], rest["onest"]

                aT = acts.tile([P, NCH, TOK], f16, tag="aT")
                for c in range(NCH):
                    bc = mmps.tile([P, TOK], f32, tag="ps")
                    nc.tensor.matmul(
                        bc, onest_sb[:, c, :], recip16, start=True, stop=True
                    )
                    nc.vector.tensor_mul(aT[:, c, :], eT[:, c, :], bc)

                o1T = acts.tile([P, NCH, TOK], f16, tag="o1T")
                for c in range(NCH):
                    ps = mmps.tile([P, TOK], f32, tag="ps")
                    for k in range(NCH):
                        nc.tensor.matmul(
                            ps,
                            wow_sb[:, k, c * P : (c + 1) * P],
                            aT[:, k, :],
                            start=(k == 0),
                            stop=(k == NCH - 1),
                        )
                    iB = bslot * 2 * NCH + 2 * c + 1
                    nc.scalar.activation(
                        o1T[:, c, :], ps, AF.Copy, accum_out=rb[:, iB : iB + 1]
                    )

                h1T = acts.tile([P, NCH, TOK], f16, tag="h1T")
                for c in range(NCH):
                    ps = mmps.tile([P, TOK], f32, tag="ps")
                    for k in range(NCH):
                        nc.tensor.matmul(
                            ps,
                            w1_sb[:, k, c * P : (c + 1) * P],
                            o1T[:, k, :],
                            start=(k == 0),
                            stop=(k == NCH - 1),
                        )
                    nc.scalar.activation(
                        h1T[:, c, :], ps, act_fn, bias=b1_sb[:, c : c + 1]
                    )

                for c in range(NCH):
                    ps = mmps.tile([P, TOK], f32, tag="ps")
                    for k in range(NCH):
                        nc.tensor.matmul(
                            ps,
                            w2_sb[:, k, c * P : (c + 1) * P],
                            h1T[:, k, :],
                            start=(k == 0),
                            stop=(k == NCH - 1),
                        )
                    iA = bslot * 2 * NCH + 2 * c
                    nc.vector.reduce_sum(rb[:, iA : iA + 1], ps, axis=AX.X)

                if bslot == s_blocks - 1:
                    bsum = smalls.tile([P, NCH], f32, tag="bsum")
                    view = rb.rearrange("p (b c t) -> p c b t", c=NCH, t=2)
                    nc.vector.reduce_sum(bsum, view, axis=AX.XY)
                    nc.vector.tensor_scalar_mul(
                        out_stage[:, bat, :], bsum, 1.0 / (s_blocks * TOK)
                    )
                    del rbs[bat]

            def whole(load_late):
                # software pipeline: front(b+1) overlaps back(b)
                for blk in range(nblocks + 1):
                    if blk < nblocks:
                        front(blk)
                    if blk == 0 and load_late:
                        do_load_rest()
                    if blk > 0:
                        back(blk - 1)
                nc.sync.dma_start(out=out_d, in_=out_stage)

            if repeat > 1:
                do_load_rest()
                with tc.For_i(0, repeat, 1):
                    whole(load_late=False)
            else:
                whole(load_late=True)

    nc.compile()
    return nc


def fold_weights(wq, kv_latent, wo, w1, b1, w2):
    """Host-side algebraic folding (float64 for accuracy)."""
    kv = kv_latent.astype(np.float64).reshape(R, N_HEADS, HD)
    blk = np.zeros((N_HEADS, HD, N_HEADS, R))
    blk2 = np.zeros((N_HEADS, R, N_HEADS, HD))
    for h in range(N_HEADS):
        blk[h, :, h, :] = kv[:, h, :].T
        blk2[h, :, h, :] = kv[:, h, :]
    Wblk = blk.reshape(D, D)
    Wblk2 = blk2.reshape(D, D)
    wqs = (wq.astype(np.float64) @ Wblk) * (R ** -0.5)
    wow = Wblk2 @ wo.astype(np.float64)

    ones_b = np.zeros((N_HEADS, R, N_HEADS))
    for h in range(N_HEADS):
        ones_b[h, :, h] = 1.0
    onesb = ones_b.reshape(D, N_HEADS)

    return {
        "wqs": np.ascontiguousarray(wqs.astype(np.float16)),
        "wow": np.ascontiguousarray(wow.astype(np.float16)),
        "w1": np.ascontiguousarray(w1.astype(np.float16)),
        "w2": np.ascontiguousarray(w2.astype(np.float16)),
        "b1t": np.ascontiguousarray(
            b1.astype(np.float32).reshape(NCH, P).T
        ),
        "onesb": np.ascontiguousarray(onesb.astype(np.float16)),
        "onest": np.ascontiguousarray(onesb.T.astype(np.float16)),
    }


_NC_CACHE = {}
LAST_RESULTS = None


def kernel(x, wq, kv_latent, wo, w1, b1, w2, b2):
    from concourse.bass_utils import run_bass_kernel_spmd

    x = np.asarray(x, dtype=np.float32)
    B, S_, D_ = x.shape
    assert D_ == D and S_ == S and B % N_CORES == 0
    nb = B // N_CORES

    shared = fold_weights(
        np.asarray(wq), np.asarray(kv_latent), np.asarray(wo),
        np.asarray(w1), np.asarray(b1), np.asarray(w2),
    )

    key = (nb,)
    if key not in _NC_CACHE:
        _NC_CACHE[key] = build_nc(nb)
    nc = _NC_CACHE[key]

    in_maps = []
    for c in range(N_CORES):
        m = dict(shared)
        m["x"] = np.ascontiguousarray(
            x[c * nb : (c + 1) * nb].reshape(nb * S, D)
        )
        in_maps.append(m)

    res = run_bass_kernel_spmd(nc, in_maps, core_ids=list(range(N_CORES)))
    global LAST_RESULTS
    LAST_RESULTS = res
    outs = []
    for r in res.results:
        outs.append(r["outT"].transpose(1, 2, 0).reshape(nb, D))
    out = np.concatenate(outs, axis=0) + np.asarray(b2, dtype=np.float32)[None, :]
    return out.astype(np.float32)
